# revision 30
# baseline (speedup 1.0000x reference)
"""Multi-head attention (B=4, S=2048, D=1024, H=16, Dk=64) on 8 trn2 cores.

Sharding: data-parallel over batch (4) x sequence-split over S (2) for the
query side. Each core computes K/V projections for its full batch element
(duplicated across the 2 cores of a batch pair) and Q/attention/Wo for its
own half of the sequence rows. Output rows are disjoint -> no collectives;
the host just concatenates the 8 [1024, 1024] slices.

Per-core kernel layout (all matmul inputs bf16, fp32 PSUM accumulation):
  x is passed pre-rotated per core so the "own" query rows are always rows
  0:1024. Attention is permutation-invariant over s_k, so K/V built from the
  rotated x give identical results.

  XT[d, s]   = x^T, transposed on host, plain DMA loads   [8 x (128, 2048)]
  QT[d', s]  = Wq^T XT (+bq)  for s in own half           [8 x (128, 1024)]
  KT[d', s]  = Wk^T XT (+bk)                              [8 x (128, 2048)]
  V'[s, hd]  = XT^T Wv (+bv via K=1 ones-matmul), stored
               per head as 65 cols: [V_h | ones] for the
               softmax denominator                        [16 x (128, 1040)]
  attention runs over head PAIRS (2t, 2t+1): the two K=64 score matmuls
  go to disjoint PE row groups (explicit tile_position) and run
  concurrently. Per pair, per 512-col s_q chunk, per s_k tile g:
    scoresT[s_k, s_q] = KT_h^T QT_h   -> one [128,1024] psum (both heads)
    expT = exp(scoresT / 8)           (ACT, psum->sbuf bf16, N=1024 ops)
    PV (lagged 2 tiles behind exp):  ctx'[65, s_q] += V'_h[g]^T expT[g]
                                      (row 64 accumulates the softmax denom)
    interleaved filler: next pair's Q/K projection matmuls (and, in pair 0,
    the V projection) keep the PE busy while ACT works through the exps
  normalize: recip = 1/ctx'[64] (DVE, fp16), broadcast across 64 partitions
    via a fp16 ones-matmul, ctxT_h = ctx'[0:64] * bcast (DVE, -> bf16)
  out[s, e] = ctxT^T Wo (+bo via K=1 ones-matmul) -> f32 -> DRAM

  Measured ~640us/core/iteration on trn2 (K-loop slope method; includes
  per-iteration input DMA + loop barrier), rel err 5.6e-3 vs fp32 reference.
"""

import sys

sys.path.insert(0, "/opt/trn_rl_repo")

import numpy as np
import ml_dtypes

import concourse.bass as bass
import concourse.bacc as bacc
import concourse.tile as tile
import concourse.mybir as mybir
from concourse.bass_utils import run_bass_kernel_spmd

BF16 = mybir.dt.bfloat16
F32 = mybir.dt.float32
F32R = mybir.dt.float32r
F16 = mybir.dt.float16

import os

NOPACK = bool(int(os.environ.get("MHA_NOPACK", "0")))
NOEXP = bool(int(os.environ.get("MHA_NOEXP", "0")))

B, S, D, H, DK = 4, 2048, 1024, 16, 64
SH = S // 2          # own-half sequence rows per core
P = 128
NT_D = D // P        # 8 tiles along d / d'
NT_S = S // P        # 16 tiles along s
NCH_Q = SH // 512    # 2 free-dim chunks for own-half s_q
NCH_S = S // 512     # 4 chunks for full s
VROW = H * (DK + 1)  # 1040: per-head 65 columns (64 V + 1 ones)


def build_kernel(loop_iters=1):
    nc = bacc.Bacc("TRN2", target_bir_lowering=False, debug=False, num_devices=8)

    x_d = nc.dram_tensor("x", [D, S], BF16, kind="ExternalInput")  # x^T, host-transposed
    wq_d = nc.dram_tensor("wq", [D, D], BF16, kind="ExternalInput")
    wk_d = nc.dram_tensor("wk", [D, D], BF16, kind="ExternalInput")
    wv_d = nc.dram_tensor("wv", [D, D], BF16, kind="ExternalInput")
    wo_d = nc.dram_tensor("wo", [D, D], BF16, kind="ExternalInput")
    bq_d = nc.dram_tensor("bq", [P, D // P], F32, kind="ExternalInput")
    bk_d = nc.dram_tensor("bk", [P, D // P], F32, kind="ExternalInput")
    bv_d = nc.dram_tensor("bv", [1, D], BF16, kind="ExternalInput")
    bo_d = nc.dram_tensor("bo", [1, D], BF16, kind="ExternalInput")
    out_d = nc.dram_tensor("out", [SH, D], F32, kind="ExternalOutput")

    with tile.TileContext(nc) as tc:
        from contextlib import ExitStack

        with ExitStack() as ctx:
            if loop_iters > 1:
                # benchmarking only: run the whole body loop_iters times in
                # one NEFF launch so per-iteration device time can be
                # measured without per-launch RPC overhead
                with tc.For_i(0, loop_iters, 1):
                    build_body(ctx, tc, nc, x_d, wq_d, wk_d, wv_d, wo_d,
                               bq_d, bk_d, bv_d, bo_d, out_d)
            else:
                build_body(ctx, tc, nc, x_d, wq_d, wk_d, wv_d, wo_d,
                           bq_d, bk_d, bv_d, bo_d, out_d)
    nc.compile()
    return nc


def build_body(ctx, tc, nc, x_d, wq_d, wk_d, wv_d, wo_d,
               bq_d, bk_d, bv_d, bo_d, out_d):
    const = ctx.enter_context(tc.tile_pool(name="const", bufs=1))
    qkv = ctx.enter_context(tc.tile_pool(name="qkv", bufs=1))
    wpool = ctx.enter_context(tc.tile_pool(name="w", bufs=1))
    xt_pool = ctx.enter_context(tc.tile_pool(name="xt", bufs=1))
    outp = ctx.enter_context(tc.tile_pool(name="outp", bufs=2))
    epool = ctx.enter_context(tc.tile_pool(name="epool", bufs=3))
    rpool = ctx.enter_context(tc.tile_pool(name="rpool", bufs=1))
    # PSUM: sp 1x4 banks + pv0/pv1 + ps x2 = 8 banks exactly
    spsum = ctx.enter_context(tc.tile_pool(name="spsum", bufs=1, space="PSUM"))
    pvpsum = ctx.enter_context(tc.tile_pool(name="pvpsum", bufs=1, space="PSUM"))
    pspsum = ctx.enter_context(tc.tile_pool(name="pspsum", bufs=2, space="PSUM"))

    # ---- constants ----
    bqs = const.tile([P, NT_D], F32, tag="bqs")
    nc.sync.dma_start(bqs[:, :], bq_d[:, :])
    bks = const.tile([P, NT_D], F32, tag="bks")
    nc.sync.dma_start(bks[:, :], bk_d[:, :])
    bvr = const.tile([1, D], BF16, tag="bvr")
    nc.sync.dma_start(bvr[:, :], bv_d[:, :])
    bor = const.tile([1, D], BF16, tag="bor")
    nc.sync.dma_start(bor[:, :], bo_d[:, :])
    # bias rows broadcast across partitions on the Pool engine: K=1
    # ones-matmuls measure ~1141 PE cycles each, a partition-replicated
    # SBUF tile + DVE add is free by comparison
    bvb = const.tile([P, D], BF16, tag="bvb")
    nc.gpsimd.partition_broadcast(bvb[:, :], bvr[:, :])
    bob = const.tile([P, D], BF16, tag="bob")
    nc.gpsimd.partition_broadcast(bob[:, :], bor[:, :])

    # ---- weights + x^T ----
    # DMA order matters: the first matmuls (Q proj m=0) need xt[k]+wq[k], then
    # K proj needs wk, then the merged V projection needs wv.
    wq = [wpool.tile([P, D], BF16, tag=f"wq{k}", name=f"wq{k}") for k in range(NT_D)]
    wk = [wpool.tile([P, D], BF16, tag=f"wk{k}", name=f"wk{k}") for k in range(NT_D)]
    wv = [wpool.tile([P, D], BF16, tag=f"wv{k}", name=f"wv{k}") for k in range(NT_D)]
    xt = [xt_pool.tile([P, S], BF16, tag=f"xt{k}", name=f"xt{k}") for k in range(NT_D)]
    for k in range(NT_D):
        nc.sync.dma_start(xt[k][:, :], x_d[k * P:(k + 1) * P, :])
        nc.sync.dma_start(wq[k][:, :], wq_d[k * P:(k + 1) * P, :])
    for k in range(NT_D):
        nc.sync.dma_start(wk[k][:, :], wk_d[k * P:(k + 1) * P, :])
    for k in range(NT_D):
        nc.sync.dma_start(wv[k][:, :], wv_d[k * P:(k + 1) * P, :])

    qt = [qkv.tile([P, SH], BF16, tag=f"qt{m}", name=f"qt{m}") for m in range(NT_D)]
    kt = [qkv.tile([P, S], BF16, tag=f"kt{m}", name=f"kt{m}") for m in range(NT_D)]
    vp = [qkv.tile([P, VROW], BF16, tag=f"vp{t}", name=f"vp{t}") for t in range(NT_S)]
    ctxT = [qkv.tile([P, SH], BF16, tag=f"ctxT{m}", name=f"ctxT{m}")
            for m in range(NT_D)]

    # ---- Q/K projection for one d'-tile m: emitted as filler closures ----
    def qk_proj_items(m):
        items = []

        def group(dst, w, chw, width, bias):
            ps = pspsum.tile([P, 512], F32, tag="ps", name=f"ps{m}{chw}")
            for k in range(NT_D):
                items.append(lambda k=k, ps=ps: nc.tensor.matmul(
                    ps[:, :],
                    lhsT=w[k][:, m * P:(m + 1) * P],
                    rhs=xt[k][:, chw * 512:(chw + 1) * 512],
                    start=(k == 0), stop=(k == NT_D - 1),
                ))
            items.append(lambda ps=ps: nc.vector.tensor_scalar_add(
                dst[:, chw * 512:(chw + 1) * 512], ps[:, :], bias[:, m:m + 1]))

        for chq in range(NCH_Q):
            group(qt[m], wq, chq, 512, bqs)
        for chk in range(NCH_S):
            group(kt[m], wk, chk, 512, bks)
        return items

    for it in qk_proj_items(0):
        it()

    # ---- V projection, one closure per s-tile (interleaved into pair 0).
    # Uses the sp pool's [128,1024] slots: alternates with pair-0 score tiles
    # so both stay double-buffered within the 4 sp banks.
    def v_proj_group(st):
        nc.vector.memset(
            vp[st].rearrange("p (h c) -> p h c", c=DK + 1)[:, :, DK:DK + 1], 1.0)
        for chv in range(2):
            ps = pspsum.tile([P, 512], F32, tag="ps", name=f"vps{st}{chv}")
            for k in range(NT_D):
                nc.tensor.matmul(
                    ps[:, :],
                    lhsT=xt[k][:, st * P:(st + 1) * P],
                    rhs=wv[k][:, chv * 512:(chv + 1) * 512],
                    start=(k == 0), stop=(k == NT_D - 1),
                )
            h0 = chv * (H // 2)
            nc.vector.tensor_add(
                vp[st].rearrange("p (h c) -> p h c", c=DK + 1)
                [:, h0:h0 + H // 2, 0:DK],
                ps.rearrange("p (h c) -> p h c", c=DK)[:, :, :],
                bvb.rearrange("p (h c) -> p h c", c=DK)[:, h0:h0 + H // 2, :],
            )

    wo = []  # loaded into wv's slots right after pair 0 (V's last wv reads)

    # ---- output projection for one s-tile: emitted as filler closures ----
    def out_proj_items(st_range):
        items = []
        for st in st_range:
            ot = outp.tile([P, D], F32, tag="ot", name=f"ot{st}")
            for cho in range(2):
                po = pspsum.tile([P, 512], F32, tag="ps", name=f"po{st}{cho}")
                for k in range(NT_D):
                    items.append(lambda k=k, po=po, st=st, cho=cho: nc.tensor.matmul(
                        po[:, :],
                        lhsT=ctxT[k][:, st * P:(st + 1) * P],
                        rhs=wo[k][:, cho * 512:(cho + 1) * 512],
                        start=(k == 0), stop=(k == NT_D - 1)))
                items.append(lambda ot=ot, po=po, cho=cho: nc.vector.tensor_add(
                    ot[:, cho * 512:(cho + 1) * 512], po[:, :],
                    bob[:, cho * 512:(cho + 1) * 512]))
            items.append(lambda st=st, ot=ot: nc.sync.dma_start(
                out_d[st * P:(st + 1) * P, :], ot[:, :]))
        return items

    # ---- attention: head pairs, pipelined scores->exp->PV with QK filler ----
    noexp_ets = None
    if NOEXP:  # timing ablation: PV reads fixed prewritten tiles, no ACT dep
        noexp_ets = [qkv.tile([P, 2048], BF16, tag=f"nxe{i}", name=f"nxe{i}")
                     for i in range(2)]
        for e in noexp_ets:
            nc.vector.memset(e[:, :], 0.5)
    # K=64 score matmuls stream far faster in alternating-row-group runs
    # uninterrupted by K=128 matmuls (~68 cyc at N=256 vs ~950 mixed-in), so
    # each head pair's scores are emitted as N=256 halves in alternating
    # bursts of 4, and the per-2g stream is [8 x score][4 x PV + fillers].
    LAG = 2
    filler, fi = [], 0
    for t in range(NT_D):  # head pair t = heads (2t, 2t+1), share kt[t]/qt[t]
        if t + 1 < NT_D:
            filler = qk_proj_items(t + 1)
            fi = 0
        for chq in range(NCH_Q):
            if t == NT_D - 1 and chq == 1:
                # projections are long done; fill the last pair's second s_q
                # chunk with the first half of the output projection (those
                # st tiles read only ctxT columns 0:512, complete after chq 0)
                while fi < len(filler):
                    filler[fi]()
                    fi += 1
                filler = out_proj_items(range(NT_D // 2))
                fi = 0
            merge_v = (t == 0 and chq == 0)
            q0 = chq * 512
            pv0 = pvpsum.tile([P, 512], F32, tag="pv0", name=f"pv0_{t}{chq}")
            pv1 = pvpsum.tile([P, 512], F32, tag="pv1", name=f"pv1_{t}{chq}")
            ets = {}

            def emit_pv(g, pv0=pv0, pv1=pv1, ets=ets, t=t):
                et = ets[g - (g % 2)]  # [P, 2048] tile holds 2 g-tiles
                base = (g % 2) * 1024
                for side, pvt in ((0, pv0), (1, pv1)):
                    h = 2 * t + side
                    nc.tensor.matmul(
                        pvt[0:DK + 1, :],
                        lhsT=vp[g][:, h * (DK + 1):(h + 1) * (DK + 1)],
                        rhs=et[:, base + side * 512:base + (side + 1) * 512],
                        start=(g == 0), stop=(g == NT_S - 1),
                    )

            # per 2-g block: one [128, 2048] psum tile (4 banks); the 8 K=64
            # N=256 score matmuls rotate over all 4 banks (adjacent matmuls
            # must hit different banks with reuse distance 4 to pipeline),
            # then ONE 2048-wide exp, then [4 x PV K=128 + fillers]
            for ge in range(0, NT_S, 2):
                if merge_v:
                    # keep V production >= LAG tiles ahead of PV consumption
                    v_proj_group(ge)
                    v_proj_group(ge + 1)
                sp = spsum.tile([P, 2048], F32, tag="sp",
                                name=f"sp{t}{chq}{ge}")
                for nh in range(2):
                    for gi, g in enumerate((ge, ge + 1)):
                        for side in range(2):
                            hoff = side * DK
                            col = gi * 1024 + side * 512 + nh * 256
                            nc.tensor.matmul(
                                sp[:, col:col + 256],
                                lhsT=kt[t][hoff:hoff + DK, g * P:(g + 1) * P],
                                rhs=qt[t][hoff:hoff + DK,
                                          q0 + nh * 256:q0 + (nh + 1) * 256],
                                start=True, stop=True,
                            )
                if NOEXP:  # timing ablation only (breaks numerics)
                    ets[ge] = noexp_ets[(ge // 2) % 2]
                else:
                    et = epool.tile([P, 2048], BF16, tag="et",
                                    name=f"et{t}{chq}{ge}")
                    ets[ge] = et
                    nc.scalar.activation(
                        et[:, :], sp[:, :],
                        mybir.ActivationFunctionType.Exp, scale=0.125,
                    )
                for gg in (ge - 2, ge - 1):
                    if gg >= 0:
                        emit_pv(gg)
                for _ in range(4):
                    if fi < len(filler):
                        filler[fi]()
                        fi += 1
            for gg in range(NT_S - LAG, NT_S):
                emit_pv(gg)

            # normalize both heads: 1/denom broadcast on the (idle) Pool engine
            for side, pvt in ((0, pv0), (1, pv1)):
                rr = rpool.tile([1, 512], F16, tag="rr", name=f"rr{t}{chq}{side}")
                with nc.allow_low_precision(reason="fp16 recip, 5e-4 rel"):
                    nc.vector.reciprocal(rr[:, :], pvt[DK:DK + 1, :])
                bcs = rpool.tile([DK, 512], F16, tag="bcs", name=f"bcs{t}{chq}{side}")
                nc.gpsimd.partition_broadcast(bcs[:, :], rr[:, :])
                nc.vector.tensor_mul(
                    ctxT[t][side * DK:(side + 1) * DK, q0:q0 + 512],
                    pvt[0:DK, :], bcs[:, :],
                )
        # make sure next pair's projections are done before its scores
        while fi < len(filler):
            filler[fi]()
            fi += 1
        if t == 0:
            # wv is dead after pair 0's merged V projection: reuse its slots
            # for wo (Tile inserts the WAR deps on the last V matmuls)
            for k in range(NT_D):
                wo.append(wpool.tile([P, D], BF16, tag=f"wv{k}", name=f"wo{k}"))
                nc.sync.dma_start(wo[k][:, :], wo_d[k * P:(k + 1) * P, :])

    # ---- output projection (second half; first half ran as pair-7 filler) ----
    for it in out_proj_items(range(NT_D // 2, NT_D)):
        it()


_NC_CACHE = None


def get_nc():
    global _NC_CACHE
    if _NC_CACHE is None:
        _NC_CACHE = build_kernel()
    return _NC_CACHE


def make_weight_map(Wq, bq, Wk, bk, Wv, bv, Wo, bo):
    """Convert the shared (all-core) weights/biases once."""
    bf = ml_dtypes.bfloat16
    return {
        "wq": np.asarray(Wq, np.float32).astype(bf),
        "wk": np.asarray(Wk, np.float32).astype(bf),
        "wv": np.asarray(Wv, np.float32).astype(bf),
        "wo": np.asarray(Wo, np.float32).astype(bf),
        "bq": np.ascontiguousarray(np.asarray(bq, np.float32).reshape(NT_D, P).T),
        "bk": np.ascontiguousarray(np.asarray(bk, np.float32).reshape(NT_D, P).T),
        "bv": np.asarray(bv, np.float32).astype(bf).reshape(1, D),
        "bo": np.asarray(bo, np.float32).astype(bf).reshape(1, D),
    }


def make_in_map(xb, Wq, bq, Wk, bk, Wv, bv, Wo, bo, _wmap=None):
    """Stage one core's inputs: xb is that core's (already rotated) [S, D]
    batch slice; weights/biases are the full fp32 tensors."""
    bf = ml_dtypes.bfloat16
    wmap = _wmap if _wmap is not None else make_weight_map(
        Wq, bq, Wk, bk, Wv, bv, Wo, bo)
    return {"x": np.ascontiguousarray(np.asarray(xb, np.float32).T.astype(bf)),
            **wmap}


def make_in_maps(x, Wq, bq, Wk, bk, Wv, bv, Wo, bo, **_):
    x = np.asarray(x, dtype=np.float32)
    wmap = make_weight_map(Wq, bq, Wk, bk, Wv, bv, Wo, bo)
    in_maps = []
    for core in range(8):
        b, half = core // 2, core % 2
        xb = x[b]
        if half == 1:
            xb = np.concatenate([xb[SH:], xb[:SH]], axis=0)
        in_maps.append(make_in_map(xb, None, None, None, None, None, None,
                                   None, None, _wmap=wmap))
    return in_maps


def kernel(x, Wq, bq, Wk, bk, Wv, bv, Wo, bo, **_):
    in_maps = make_in_maps(x, Wq, bq, Wk, bk, Wv, bv, Wo, bo)

    nc = get_nc()
    res = run_bass_kernel_spmd(nc, in_maps, core_ids=list(range(8)))
    out = np.empty((B, S, D), dtype=np.float32)
    for core in range(8):
        b, half = core // 2, core % 2
        out[b, half * SH:(half + 1) * SH] = res.results[core]["out"]
    return out


if __name__ == "__main__":
    rng = np.random.default_rng(0)
    ins = {
        "x": rng.standard_normal((B, S, D), dtype=np.float32),
        "Wq": rng.standard_normal((D, D), dtype=np.float32) * 0.03,
        "bq": rng.standard_normal(D).astype(np.float32) * 0.01,
        "Wk": rng.standard_normal((D, D), dtype=np.float32) * 0.03,
        "bk": rng.standard_normal(D).astype(np.float32) * 0.01,
        "Wv": rng.standard_normal((D, D), dtype=np.float32) * 0.03,
        "bv": rng.standard_normal(D).astype(np.float32) * 0.01,
        "Wo": rng.standard_normal((D, D), dtype=np.float32) * 0.03,
        "bo": rng.standard_normal(D).astype(np.float32) * 0.01,
    }
    y = kernel(**ins)
    print(y.shape, y.dtype, float(np.abs(y).max()))



# revision 32
# speedup vs baseline: 1.2474x; 1.2474x over previous
"""Multi-head attention (B=4, S=2048, D=1024, H=16, Dk=64) on 8 trn2 cores.

Sharding: data-parallel over batch (4) x sequence-split over S (2) for the
query side. Each core computes K/V projections for its full batch element
(duplicated across the 2 cores of a batch pair) and Q/attention/Wo for its
own half of the sequence rows. Output rows are disjoint -> no collectives;
the host just concatenates the 8 [1024, 1024] slices.

Per-core kernel layout (all matmul inputs bf16, fp32 PSUM accumulation):
  x is passed pre-rotated per core so the "own" query rows are always rows
  0:1024. Attention is permutation-invariant over s_k, so K/V built from the
  rotated x give identical results.

  XT[d, s]   = x^T, transposed on host, plain DMA loads   [8 x (128, 2048)]
  QT[d', s]  = Wq^T XT (+bq)  for s in own half           [8 x (128, 1024)]
  KT[d', s]  = Wk^T XT (+bk)                              [8 x (128, 2048)]
  V'[s, hd]  = XT^T Wv (+bv via K=1 ones-matmul), stored
               per head as 65 cols: [V_h | ones] for the
               softmax denominator                        [16 x (128, 1040)]
  attention runs over head PAIRS (2t, 2t+1): the two K=64 score matmuls
  go to disjoint PE row groups (explicit tile_position) and run
  concurrently. Per pair, per 512-col s_q chunk, per s_k tile g:
    scoresT[s_k, s_q] = KT_h^T QT_h   -> one [128,1024] psum (both heads)
    expT = exp(scoresT / 8)           (ACT, psum->sbuf bf16, N=1024 ops)
    PV (lagged 2 tiles behind exp):  ctx'[65, s_q] += V'_h[g]^T expT[g]
                                      (row 64 accumulates the softmax denom)
    interleaved filler: next pair's Q/K projection matmuls (and, in pair 0,
    the V projection) keep the PE busy while ACT works through the exps
  normalize: recip = 1/ctx'[64] (DVE, fp16), broadcast across 64 partitions
    via a fp16 ones-matmul, ctxT_h = ctx'[0:64] * bcast (DVE, -> bf16)
  out[s, e] = ctxT^T Wo (+bo via K=1 ones-matmul) -> f32 -> DRAM

  Measured ~640us/core/iteration on trn2 (K-loop slope method; includes
  per-iteration input DMA + loop barrier), rel err 5.6e-3 vs fp32 reference.
"""

import sys

sys.path.insert(0, "/opt/trn_rl_repo")

import numpy as np
import ml_dtypes

import concourse.bass as bass
import concourse.bacc as bacc
import concourse.tile as tile
import concourse.mybir as mybir
from concourse.bass_utils import run_bass_kernel_spmd

BF16 = mybir.dt.bfloat16
F32 = mybir.dt.float32
F32R = mybir.dt.float32r
F16 = mybir.dt.float16

import os

NOPACK = bool(int(os.environ.get("MHA_NOPACK", "0")))
NOEXP = bool(int(os.environ.get("MHA_NOEXP", "0")))

B, S, D, H, DK = 4, 2048, 1024, 16, 64
SH = S // 2          # own-half sequence rows per core
P = 128
NT_D = D // P        # 8 tiles along d / d'
NT_S = S // P        # 16 tiles along s
NCH_Q = SH // 512    # 2 free-dim chunks for own-half s_q
NCH_S = S // 512     # 4 chunks for full s
VROW = H * (DK + 1)  # 1040: per-head 65 columns (64 V + 1 ones)


def build_kernel(loop_iters=1):
    nc = bacc.Bacc("TRN2", target_bir_lowering=False, debug=False, num_devices=8)

    x_d = nc.dram_tensor("x", [D, S], BF16, kind="ExternalInput")  # x^T, host-transposed
    wq_d = nc.dram_tensor("wq", [D, D], BF16, kind="ExternalInput")
    wk_d = nc.dram_tensor("wk", [D, D], BF16, kind="ExternalInput")
    wv_d = nc.dram_tensor("wv", [D, D], BF16, kind="ExternalInput")
    wo_d = nc.dram_tensor("wo", [D, D], BF16, kind="ExternalInput")
    bq_d = nc.dram_tensor("bq", [P, D // P], F32, kind="ExternalInput")
    bk_d = nc.dram_tensor("bk", [P, D // P], F32, kind="ExternalInput")
    bv_d = nc.dram_tensor("bv", [1, D], BF16, kind="ExternalInput")
    bo_d = nc.dram_tensor("bo", [1, D], BF16, kind="ExternalInput")
    out_d = nc.dram_tensor("out", [SH, D], F32, kind="ExternalOutput")

    with tile.TileContext(nc) as tc:
        from contextlib import ExitStack

        with ExitStack() as ctx:
            if loop_iters > 1:
                # benchmarking only: run the whole body loop_iters times in
                # one NEFF launch so per-iteration device time can be
                # measured without per-launch RPC overhead
                with tc.For_i(0, loop_iters, 1):
                    build_body(ctx, tc, nc, x_d, wq_d, wk_d, wv_d, wo_d,
                               bq_d, bk_d, bv_d, bo_d, out_d)
            else:
                build_body(ctx, tc, nc, x_d, wq_d, wk_d, wv_d, wo_d,
                           bq_d, bk_d, bv_d, bo_d, out_d)
    nc.compile()
    return nc


def build_body(ctx, tc, nc, x_d, wq_d, wk_d, wv_d, wo_d,
               bq_d, bk_d, bv_d, bo_d, out_d):
    const = ctx.enter_context(tc.tile_pool(name="const", bufs=1))
    qkv = ctx.enter_context(tc.tile_pool(name="qkv", bufs=1))
    wpool = ctx.enter_context(tc.tile_pool(name="w", bufs=1))
    xt_pool = ctx.enter_context(tc.tile_pool(name="xt", bufs=1))
    outp = ctx.enter_context(tc.tile_pool(name="outp", bufs=2))
    epool = ctx.enter_context(tc.tile_pool(name="epool", bufs=5))
    rpool = ctx.enter_context(tc.tile_pool(name="rpool", bufs=2))
    # PSUM: sp 2x2 banks + pv0/pv1 + ps x2 = 8 banks exactly
    spsum = ctx.enter_context(tc.tile_pool(name="spsum", bufs=2, space="PSUM"))
    pvpsum = ctx.enter_context(tc.tile_pool(name="pvpsum", bufs=1, space="PSUM"))
    pspsum = ctx.enter_context(tc.tile_pool(name="pspsum", bufs=2, space="PSUM"))

    # ---- constants ----
    bqs = const.tile([P, NT_D], F32, tag="bqs")
    nc.sync.dma_start(bqs[:, :], bq_d[:, :])
    bks = const.tile([P, NT_D], F32, tag="bks")
    nc.sync.dma_start(bks[:, :], bk_d[:, :])
    bvr = const.tile([1, D], BF16, tag="bvr")
    nc.sync.dma_start(bvr[:, :], bv_d[:, :])
    bor = const.tile([1, D], BF16, tag="bor")
    nc.sync.dma_start(bor[:, :], bo_d[:, :])
    # bias rows broadcast across partitions on the Pool engine: K=1
    # ones-matmuls measure ~1141 PE cycles each, a partition-replicated
    # SBUF tile + DVE add is free by comparison
    bvb = const.tile([P, D], BF16, tag="bvb")
    nc.gpsimd.partition_broadcast(bvb[:, :], bvr[:, :])
    bob = const.tile([P, D], BF16, tag="bob")
    nc.gpsimd.partition_broadcast(bob[:, :], bor[:, :])

    # ---- weights + x^T ----
    # DMA order matters: the first matmuls (Q proj m=0) need xt[k]+wq[k], then
    # K proj needs wk, then the merged V projection needs wv.
    wq = [wpool.tile([P, D], BF16, tag=f"wq{k}", name=f"wq{k}") for k in range(NT_D)]
    wk = [wpool.tile([P, D], BF16, tag=f"wk{k}", name=f"wk{k}") for k in range(NT_D)]
    wv = [wpool.tile([P, D], BF16, tag=f"wv{k}", name=f"wv{k}") for k in range(NT_D)]
    xt = [xt_pool.tile([P, S], BF16, tag=f"xt{k}", name=f"xt{k}") for k in range(NT_D)]
    for k in range(NT_D):
        nc.sync.dma_start(xt[k][:, :], x_d[k * P:(k + 1) * P, :])
        nc.sync.dma_start(wq[k][:, :], wq_d[k * P:(k + 1) * P, :])
    for k in range(NT_D):
        nc.sync.dma_start(wk[k][:, :], wk_d[k * P:(k + 1) * P, :])
    for k in range(NT_D):
        nc.sync.dma_start(wv[k][:, :], wv_d[k * P:(k + 1) * P, :])

    qt = [qkv.tile([P, SH], BF16, tag=f"qt{m}", name=f"qt{m}") for m in range(NT_D)]
    kt = [qkv.tile([P, S], BF16, tag=f"kt{m}", name=f"kt{m}") for m in range(NT_D)]
    vp = [qkv.tile([P, VROW], BF16, tag=f"vp{t}", name=f"vp{t}") for t in range(NT_S)]
    ctxT = [qkv.tile([P, SH], BF16, tag=f"ctxT{m}", name=f"ctxT{m}")
            for m in range(NT_D)]

    # ---- Q/K projection for one d'-tile m: emitted as filler closures ----
    def qk_proj_items(m):
        items = []

        def group(dst, w, chw, width, bias):
            ps = pspsum.tile([P, 512], F32, tag="ps", name=f"ps{m}{chw}")
            for k in range(NT_D):
                items.append(lambda k=k, ps=ps: nc.tensor.matmul(
                    ps[:, :],
                    lhsT=w[k][:, m * P:(m + 1) * P],
                    rhs=xt[k][:, chw * 512:(chw + 1) * 512],
                    start=(k == 0), stop=(k == NT_D - 1),
                ))
            items.append(lambda ps=ps: nc.vector.tensor_scalar_add(
                dst[:, chw * 512:(chw + 1) * 512], ps[:, :], bias[:, m:m + 1]))

        for chq in range(NCH_Q):
            group(qt[m], wq, chq, 512, bqs)
        for chk in range(NCH_S):
            group(kt[m], wk, chk, 512, bks)
        return items

    for it in qk_proj_items(0):
        it()

    # ---- V projection, one closure per s-tile (interleaved into pair 0).
    # Uses the sp pool's [128,1024] slots: alternates with pair-0 score tiles
    # so both stay double-buffered within the 4 sp banks.
    def v_proj_group(st):
        nc.vector.memset(
            vp[st].rearrange("p (h c) -> p h c", c=DK + 1)[:, :, DK:DK + 1], 1.0)
        ps = spsum.tile([P, 1024], F32, tag="sp", name=f"vps{st}")
        for chv in range(2):
            half = ps[:, chv * 512:(chv + 1) * 512]
            for k in range(NT_D):
                nc.tensor.matmul(
                    half,
                    lhsT=xt[k][:, st * P:(st + 1) * P],
                    rhs=wv[k][:, chv * 512:(chv + 1) * 512],
                    start=(k == 0), stop=(k == NT_D - 1),
                )
        nc.vector.tensor_add(
            vp[st].rearrange("p (h c) -> p h c", c=DK + 1)[:, :, 0:DK],
            ps.rearrange("p (h c) -> p h c", c=DK)[:, :, :],
            bvb.rearrange("p (h c) -> p h c", c=DK)[:, :, :],
        )

    wo = []  # loaded into wv's slots right after pair 0 (V's last wv reads)

    # ---- output projection for one s-tile: emitted as filler closures ----
    def out_proj_items(st_range):
        items = []
        for st in st_range:
            ot = outp.tile([P, D], F32, tag="ot", name=f"ot{st}")
            for cho in range(2):
                po = pspsum.tile([P, 512], F32, tag="ps", name=f"po{st}{cho}")
                for k in range(NT_D):
                    items.append(lambda k=k, po=po, st=st, cho=cho: nc.tensor.matmul(
                        po[:, :],
                        lhsT=ctxT[k][:, st * P:(st + 1) * P],
                        rhs=wo[k][:, cho * 512:(cho + 1) * 512],
                        start=(k == 0), stop=(k == NT_D - 1)))
                items.append(lambda ot=ot, po=po, cho=cho: nc.vector.tensor_add(
                    ot[:, cho * 512:(cho + 1) * 512], po[:, :],
                    bob[:, cho * 512:(cho + 1) * 512]))
            items.append(lambda st=st, ot=ot: nc.sync.dma_start(
                out_d[st * P:(st + 1) * P, :], ot[:, :]))
        return items

    # ---- attention: head pairs, pipelined scores->exp->PV with QK filler ----
    noexp_ets = None
    if NOEXP:  # timing ablation: PV reads fixed prewritten tiles, no ACT dep
        noexp_ets = [qkv.tile([P, 1024], BF16, tag=f"nxe{i}", name=f"nxe{i}")
                     for i in range(2)]
        for e in noexp_ets:
            nc.vector.memset(e[:, :], 0.5)
    # K=64 score matmuls stream far faster in alternating-row-group runs
    # uninterrupted by K=128 matmuls (~68 cyc at N=256 vs ~950 mixed-in), so
    # each head pair's scores are emitted as N=256 halves in alternating
    # bursts of 4, and the per-2g stream is [8 x score][4 x PV + fillers].
    LAG = 2
    filler, fi = [], 0
    for t in range(NT_D):  # head pair t = heads (2t, 2t+1), share kt[t]/qt[t]
        if t + 1 < NT_D:
            filler = qk_proj_items(t + 1)
            fi = 0
        for chq in range(NCH_Q):
            if t == NT_D - 1 and chq == 1:
                # projections are long done; fill the last pair's second s_q
                # chunk with the first half of the output projection (those
                # st tiles read only ctxT columns 0:512, complete after chq 0)
                while fi < len(filler):
                    filler[fi]()
                    fi += 1
                filler = out_proj_items(range(NT_D // 2))
                fi = 0
            merge_v = (t == 0 and chq == 0)
            q0 = chq * 512
            pv0 = pvpsum.tile([P, 512], F32, tag="pv0", name=f"pv0_{t}{chq}")
            pv1 = pvpsum.tile([P, 512], F32, tag="pv1", name=f"pv1_{t}{chq}")
            ets = {}

            def emit_pv(g, pv0=pv0, pv1=pv1, ets=ets, t=t):
                for side, pvt in ((0, pv0), (1, pv1)):
                    h = 2 * t + side
                    nc.tensor.matmul(
                        pvt[0:DK + 1, :],
                        lhsT=vp[g][:, h * (DK + 1):(h + 1) * (DK + 1)],
                        rhs=ets[g][:, side * 512:(side + 1) * 512],
                        start=(g == 0), stop=(g == NT_S - 1),
                    )

            # per 2-g block the PE stream is [4 x score K=64][4 x PV K=128
            # + fillers]; deeper K=64 bursts are blocked by PSUM capacity
            # (scores live in psum until exp drains them)
            for g in range(NT_S):
                if merge_v:
                    # keep V production >= LAG tiles ahead of PV consumption
                    v_proj_group(g)
                sp = spsum.tile([P, 1024], F32, tag="sp", name=f"sp{t}{chq}{g}")
                for side in range(2):
                    hoff = side * DK
                    nc.tensor.matmul(
                        sp[:, side * 512:(side + 1) * 512],
                        lhsT=kt[t][hoff:hoff + DK, g * P:(g + 1) * P],
                        rhs=qt[t][hoff:hoff + DK, q0:q0 + 512],
                        start=True, stop=True,
                    )
                if NOEXP:  # timing ablation only (breaks numerics)
                    ets[g] = noexp_ets[g % 2]
                else:
                    et = epool.tile([P, 1024], BF16, tag="et",
                                    name=f"et{t}{chq}{g}")
                    ets[g] = et
                    nc.scalar.activation(
                        et[:, :], sp[:, :],
                        mybir.ActivationFunctionType.Exp, scale=0.125,
                    )
                if g % 2 == 1:
                    for gg in (g - 3, g - 2):
                        if gg >= 0:
                            emit_pv(gg)
                    for _ in range(4):
                        if fi < len(filler):
                            filler[fi]()
                            fi += 1
            for gg in range(NT_S - LAG, NT_S):
                emit_pv(gg)

            # normalize both heads: 1/denom broadcast on the (idle) Pool engine
            for side, pvt in ((0, pv0), (1, pv1)):
                rr = rpool.tile([1, 512], F16, tag="rr", name=f"rr{t}{chq}{side}")
                with nc.allow_low_precision(reason="fp16 recip, 5e-4 rel"):
                    nc.vector.reciprocal(rr[:, :], pvt[DK:DK + 1, :])
                bcs = rpool.tile([DK, 512], F16, tag="bcs", name=f"bcs{t}{chq}{side}")
                nc.gpsimd.partition_broadcast(bcs[:, :], rr[:, :])
                nc.vector.tensor_mul(
                    ctxT[t][side * DK:(side + 1) * DK, q0:q0 + 512],
                    pvt[0:DK, :], bcs[:, :],
                )
        # make sure next pair's projections are done before its scores
        while fi < len(filler):
            filler[fi]()
            fi += 1
        if t == 0:
            # wv is dead after pair 0's merged V projection: reuse its slots
            # for wo (Tile inserts the WAR deps on the last V matmuls)
            for k in range(NT_D):
                wo.append(wpool.tile([P, D], BF16, tag=f"wv{k}", name=f"wo{k}"))
                nc.sync.dma_start(wo[k][:, :], wo_d[k * P:(k + 1) * P, :])

    # ---- output projection (second half; first half ran as pair-7 filler) ----
    for it in out_proj_items(range(NT_D // 2, NT_D)):
        it()


_NC_CACHE = None


def get_nc():
    global _NC_CACHE
    if _NC_CACHE is None:
        _NC_CACHE = build_kernel()
    return _NC_CACHE


def make_weight_map(Wq, bq, Wk, bk, Wv, bv, Wo, bo):
    """Convert the shared (all-core) weights/biases once."""
    bf = ml_dtypes.bfloat16
    return {
        "wq": np.asarray(Wq, np.float32).astype(bf),
        "wk": np.asarray(Wk, np.float32).astype(bf),
        "wv": np.asarray(Wv, np.float32).astype(bf),
        "wo": np.asarray(Wo, np.float32).astype(bf),
        "bq": np.ascontiguousarray(np.asarray(bq, np.float32).reshape(NT_D, P).T),
        "bk": np.ascontiguousarray(np.asarray(bk, np.float32).reshape(NT_D, P).T),
        "bv": np.asarray(bv, np.float32).astype(bf).reshape(1, D),
        "bo": np.asarray(bo, np.float32).astype(bf).reshape(1, D),
    }


def make_in_map(xb, Wq, bq, Wk, bk, Wv, bv, Wo, bo, _wmap=None):
    """Stage one core's inputs: xb is that core's (already rotated) [S, D]
    batch slice; weights/biases are the full fp32 tensors."""
    bf = ml_dtypes.bfloat16
    wmap = _wmap if _wmap is not None else make_weight_map(
        Wq, bq, Wk, bk, Wv, bv, Wo, bo)
    return {"x": np.ascontiguousarray(np.asarray(xb, np.float32).T.astype(bf)),
            **wmap}


def make_in_maps(x, Wq, bq, Wk, bk, Wv, bv, Wo, bo, **_):
    x = np.asarray(x, dtype=np.float32)
    wmap = make_weight_map(Wq, bq, Wk, bk, Wv, bv, Wo, bo)
    in_maps = []
    for core in range(8):
        b, half = core // 2, core % 2
        xb = x[b]
        if half == 1:
            xb = np.concatenate([xb[SH:], xb[:SH]], axis=0)
        in_maps.append(make_in_map(xb, None, None, None, None, None, None,
                                   None, None, _wmap=wmap))
    return in_maps


def kernel(x, Wq, bq, Wk, bk, Wv, bv, Wo, bo, **_):
    in_maps = make_in_maps(x, Wq, bq, Wk, bk, Wv, bv, Wo, bo)

    nc = get_nc()
    res = run_bass_kernel_spmd(nc, in_maps, core_ids=list(range(8)))
    out = np.empty((B, S, D), dtype=np.float32)
    for core in range(8):
        b, half = core // 2, core % 2
        out[b, half * SH:(half + 1) * SH] = res.results[core]["out"]
    return out


if __name__ == "__main__":
    rng = np.random.default_rng(0)
    ins = {
        "x": rng.standard_normal((B, S, D), dtype=np.float32),
        "Wq": rng.standard_normal((D, D), dtype=np.float32) * 0.03,
        "bq": rng.standard_normal(D).astype(np.float32) * 0.01,
        "Wk": rng.standard_normal((D, D), dtype=np.float32) * 0.03,
        "bk": rng.standard_normal(D).astype(np.float32) * 0.01,
        "Wv": rng.standard_normal((D, D), dtype=np.float32) * 0.03,
        "bv": rng.standard_normal(D).astype(np.float32) * 0.01,
        "Wo": rng.standard_normal((D, D), dtype=np.float32) * 0.03,
        "bo": rng.standard_normal(D).astype(np.float32) * 0.01,
    }
    y = kernel(**ins)
    print(y.shape, y.dtype, float(np.abs(y).max()))

